# revision 10
# baseline (speedup 1.0000x reference)
"""Trainium2 Bass kernel for nn_Actor (dense MLP trunk + 64 softmax heads).

Data-parallel over 8 NeuronCores: batch 4096 -> 512 rows/core, weights
replicated. Feature-major trunk (activations [features, batch]) so layer
outputs feed the next contraction without transposes; heads run batch-major
so per-head softmax reduces along the free dim.

Precision split: trunk matmuls in bf16 (moving operand streams 2 elem/cycle,
LDWEIGHTS hidden, half the weight DMA), head matmuls in float32r (~1.5e-4)
with the stationary h2 tile reused across 4 column chunks (k-outer loop).

Self-contained: hardcodes shapes; host-side prep packs head weights into one
[1024, 1280] GEMM whose columns are already in the final output order
(per vehicle v: rsu[2v] | rsu[2v+1] | lay[2v] | lay[2v+1]); head bias is
folded in multiplicatively via exp(bias).
"""

import os
import numpy as np

B, IN_DIM, HIDDEN, H2 = 4096, 2048, 2048, 1024
V, R, L = 16, 32, 8
OUTC = V * (2 * R + 2 * L)          # 1280
NCORES = 8
BC = B // NCORES                    # 512 batch rows per core
KT1 = IN_DIM // 128                 # 16 k-tiles, layer 1
MT1 = HIDDEN // 128                 # 16 m-tiles, layer 1
KT2 = HIDDEN // 128                 # 16 k-tiles, layer 2
MT2 = H2 // 128                     # 8 m-tiles, layer 2
KTH = H2 // 128                     # 8 k-tiles, heads
BT = BC // 128                      # 4 batch tiles per core
CW = 320                            # head chunk width = 4 vehicles
NCH = OUTC // CW                    # 4 chunks
VC = CW // (2 * (R + L))            # 4 vehicles per chunk

_CACHE = {}
LAST_RESULTS = None                 # BassKernelResults from the last run


def _build():
    import bass_rust
    import concourse.bacc as bacc
    import concourse.mybir as mybir
    import concourse.tile as tile

    F32, F32R = mybir.dt.float32, mybir.dt.float32r
    BF16 = mybir.dt.bfloat16
    Relu = mybir.ActivationFunctionType.Relu
    Exp = mybir.ActivationFunctionType.Exp
    Copy = mybir.ActivationFunctionType.Copy
    X = mybir.AxisListType.X

    nc = bacc.Bacc("TRN2", target_bir_lowering=False, debug=False,
                   num_devices=NCORES)

    xt = nc.dram_tensor("xt", [128, KT1, BC], BF16, kind="ExternalInput")
    w1t = nc.dram_tensor("w1t", [MT1, 128, KT1, 128], BF16, kind="ExternalInput")
    b1c = nc.dram_tensor("b1c", [128, MT1], F32, kind="ExternalInput")
    w2t = nc.dram_tensor("w2t", [MT2, 128, KT2, 128], BF16, kind="ExternalInput")
    b2c = nc.dram_tensor("b2c", [128, MT2], F32, kind="ExternalInput")
    wht = nc.dram_tensor("wht", [128, KTH, OUTC], F32, kind="ExternalInput")
    ebr = nc.dram_tensor("ebr", [128, OUTC], F32, kind="ExternalInput")
    out = nc.dram_tensor("out", [BC, OUTC], F32, kind="ExternalOutput")

    with tile.TileContext(nc) as tc:
        with (
            tc.tile_pool(name="const", bufs=1) as cp,
            tc.tile_pool(name="wpool", bufs=6) as wp,
            tc.tile_pool(name="sm", bufs=3) as sp,
            tc.tile_pool(name="ps", bufs=4, space="PSUM") as ps,
            tc.tile_pool(name="psh", bufs=2, space="PSUM") as psh,
        ):
            xt_sb = cp.tile([128, KT1, BC], BF16, tag="xt")
            h1_sb = cp.tile([128, MT1, BC], BF16, tag="h1")
            h2_sb = cp.tile([128, MT2, BC], F32R, tag="h2")
            wh_sb = cp.tile([128, KTH, OUTC], F32R, tag="wh")
            eb_sb = cp.tile([128, OUTC], F32, tag="eb")
            b1_sb = cp.tile([128, MT1], F32, tag="b1")
            b2_sb = cp.tile([128, MT2], F32, tag="b2")

            # One HWDGE ring (sync), loads in exact consumption order:
            # biases, then w1[0] quarters interleaved with xt quads so the
            # PE starts after ~300KB and never waits long for either stream.
            nc.sync.dma_start(b1_sb[:], b1c.ap())
            nc.sync.dma_start(b2_sb[:], b2c.ap())
            w1m0 = wp.tile([128, KT1, 128], BF16, tag="wm")
            for q in range(4):
                nc.sync.dma_start(w1m0[:, 4 * q:4 * (q + 1), :],
                                  w1t.ap()[0][:, 4 * q:4 * (q + 1), :])
                for k in range(4 * q, 4 * q + 4):
                    nc.sync.dma_start(xt_sb[:, k, :], xt.ap()[:, k, :])

            # Layer 1: h1[m] = relu(sum_k w1[k,m].T @ xt[k] + b1[m])
            # w1 m-slices stream on the sync ring; first slice split so the
            # PE can start after ~128KB.
            for m in range(MT1):
                if m == 0:
                    w1m = w1m0
                else:
                    w1m = wp.tile([128, KT1, 128], BF16, tag="wm")
                    nc.sync.dma_start(w1m[:], w1t.ap()[m])
                acc = ps.tile([128, 512], F32, tag="acc")
                for k in range(KT1):
                    nc.tensor.matmul(acc[:], w1m[:, k, :], xt_sb[:, k, :],
                                     start=(k == 0), stop=(k == KT1 - 1))
                relu = nc.scalar.activation(h1_sb[:, m, :], acc[:], Relu,
                                            bias=b1_sb[:, m:m + 1], scale=1.0)
                # wh prefetch paced to L1 progress so it can't starve the
                # xt/w1 streams of HBM bandwidth at kernel start
                if m >= 2 and m % 2 == 0:
                    kh = (m - 2) // 2
                    whd = nc.gpsimd.dma_start(wh_sb[:, kh, :],
                                              wht.ap()[:, kh, :].bitcast(F32R))
                    bass_rust.add_dep_helper(whd.ins, relu.ins, sync=True,
                                             reason="pace wh prefetch")
                if m == 15:
                    ebd = nc.gpsimd.dma_start(eb_sb[:], ebr.ap())
                    bass_rust.add_dep_helper(ebd.ins, relu.ins, sync=True,
                                             reason="pace eb prefetch")

            # Layer 2: h2[m] = relu(sum_k w2[k,m].T @ h1[k] + b2[m])
            for m in range(MT2):
                w2m = wp.tile([128, KT2, 128], BF16, tag="wm")
                nc.sync.dma_start(w2m[:], w2t.ap()[m])
                acc = ps.tile([128, 512], F32, tag="acc")
                for k in range(KT2):
                    nc.tensor.matmul(acc[:], w2m[:, k, :], h1_sb[:, k, :],
                                     start=(k == 0), stop=(k == KT2 - 1))
                relu = nc.scalar.activation(h2_sb[:, m, :], acc[:], Relu,
                                            bias=b2_sb[:, m:m + 1], scale=1.0)
                if m == 0:
                    whd = nc.gpsimd.dma_start(wh_sb[:, KTH - 1, :],
                                              wht.ap()[:, KTH - 1, :].bitcast(F32R))
                    bass_rust.add_dep_helper(whd.ins, relu.ins, sync=True,
                                             reason="pace wh prefetch")

            # Heads: logits[b, :] = h2[:, b].T @ wh, f32r, k-outer so each
            # stationary h2 tile is loaded once and reused for all 4 chunks.
            # softmax(l + bias) == exp(l)*exp(bias) / sum(exp(l)*exp(bias))
            for bt in range(BT):
                bsl = slice(bt * 128, (bt + 1) * 128)
                for pr in range(NCH // 2):
                    accs = []
                    for ci in range(2):
                        hacc = psh.tile([128, CW], F32, tag=f"hacc{ci}")
                        accs.append(hacc)
                    for k in range(KTH):
                        for ci in range(2):
                            c = 2 * pr + ci
                            nc.tensor.matmul(accs[ci][:], h2_sb[:, k, bsl],
                                             wh_sb[:, k, c * CW:(c + 1) * CW],
                                             start=(k == 0), stop=(k == KTH - 1))
                    for ci in range(2):
                        c = 2 * pr + ci
                        c0 = c * CW
                        et = sp.tile([128, CW], F32, tag="et")
                        nc.scalar.activation(et[:], accs[ci][:], Exp)
                        num = sp.tile([128, CW], F32, tag="num")
                        nc.gpsimd.tensor_mul(num[:], et[:], eb_sb[:, c0:c0 + CW])

                        nv = num[:].rearrange("p (v x) -> p v x", v=VC)
                        rsu4 = nv[:, :, 0:2 * R].rearrange("p v (h c) -> p v h c", h=2)
                        lay4 = nv[:, :, 2 * R:].rearrange("p v (h c) -> p v h c", h=2)
                        sums = sp.tile([128, 4 * VC], F32, tag="sums")
                        s_r = sums[:, 0:2 * VC].rearrange("p (v h) -> p v h", h=2)
                        s_l = sums[:, 2 * VC:].rearrange("p (v h) -> p v h", h=2)
                        nc.vector.reduce_sum(out=s_r.unsqueeze(3), in_=rsu4, axis=X)
                        nc.vector.reduce_sum(out=s_l.unsqueeze(3), in_=lay4, axis=X)
                        rec = sp.tile([128, 4 * VC], F32, tag="rec")
                        nc.vector.reciprocal(rec[:], sums[:])

                        # expand per-block reciprocals to a per-column tile,
                        # then one contiguous multiply
                        rex = sp.tile([128, CW], F32, tag="rex")
                        xv = rex[:].rearrange("p (v x) -> p v x", v=VC)
                        xrsu = xv[:, :, 0:2 * R].rearrange("p v (h c) -> p v h c", h=2)
                        xlay = xv[:, :, 2 * R:].rearrange("p v (h c) -> p v h c", h=2)
                        r_r = rec[:, 0:2 * VC].rearrange("p (v h) -> p v h", h=2)
                        r_l = rec[:, 2 * VC:].rearrange("p (v h) -> p v h", h=2)
                        nc.scalar.activation(
                            xrsu, r_r.unsqueeze(3).broadcast_to([128, VC, 2, R]), Copy)
                        nc.gpsimd.tensor_copy(
                            xlay, r_l.unsqueeze(3).broadcast_to([128, VC, 2, L]))
                        o_sb = sp.tile([128, CW], F32, tag="o")
                        nc.vector.tensor_mul(o_sb[:], num[:], rex[:])
                        nc.sync.dma_start(out.ap()[bsl, c0:c0 + CW], o_sb[:])

    nc.compile()
    return nc


def _prep_shared(w1, b1, w2, b2, w_rsu, b_rsu, w_lay, b_lay):
    import ml_dtypes
    f, bf = np.float32, ml_dtypes.bfloat16
    w1t = np.ascontiguousarray(
        w1.astype(bf).reshape(KT1, 128, MT1, 128).transpose(2, 1, 0, 3))
    w2t = np.ascontiguousarray(
        w2.astype(bf).reshape(KT2, 128, MT2, 128).transpose(2, 1, 0, 3))
    b1c = np.ascontiguousarray(b1.reshape(MT1, 128).T, dtype=f)
    b2c = np.ascontiguousarray(b2.reshape(MT2, 128).T, dtype=f)

    wh = np.empty((H2, OUTC), dtype=f)
    bh = np.empty((OUTC,), dtype=f)
    for v in range(V):
        c = 2 * (R + L) * v
        wh[:, c:c + R] = w_rsu[2 * v]
        wh[:, c + R:c + 2 * R] = w_rsu[2 * v + 1]
        wh[:, c + 2 * R:c + 2 * R + L] = w_lay[2 * v]
        wh[:, c + 2 * R + L:c + 2 * (R + L)] = w_lay[2 * v + 1]
        bh[c:c + R] = b_rsu[2 * v]
        bh[c + R:c + 2 * R] = b_rsu[2 * v + 1]
        bh[c + 2 * R:c + 2 * R + L] = b_lay[2 * v]
        bh[c + 2 * R + L:c + 2 * (R + L)] = b_lay[2 * v + 1]
    wht = np.ascontiguousarray(
        wh.reshape(KTH, 128, OUTC).transpose(1, 0, 2), dtype=f)
    ebr = np.ascontiguousarray(
        np.broadcast_to(np.exp(bh)[None, :], (128, OUTC)), dtype=f)
    return {"w1t": w1t, "b1c": b1c, "w2t": w2t, "b2c": b2c,
            "wht": wht, "ebr": ebr}


def kernel(x, w1, b1, w2, b2, w_rsu, b_rsu, w_lay, b_lay):
    global LAST_RESULTS
    import ml_dtypes
    from concourse.bass_utils import run_bass_kernel_spmd

    if "nc" not in _CACHE:
        _CACHE["nc"] = _build()
    nc = _CACHE["nc"]

    shared = _prep_shared(np.asarray(w1, np.float32), np.asarray(b1, np.float32),
                          np.asarray(w2, np.float32), np.asarray(b2, np.float32),
                          np.asarray(w_rsu, np.float32), np.asarray(b_rsu, np.float32),
                          np.asarray(w_lay, np.float32), np.asarray(b_lay, np.float32))

    # x [B, IN] -> per-core xt [128, KT1, BC] with [p, k, n] = x[core*BC+n, k*128+p]
    xt_full = np.ascontiguousarray(np.asarray(x, np.float32).T) \
        .astype(ml_dtypes.bfloat16).reshape(KT1, 128, B).transpose(1, 0, 2)
    in_maps = []
    for c in range(NCORES):
        m = dict(shared)
        m["xt"] = np.ascontiguousarray(xt_full[:, :, c * BC:(c + 1) * BC])
        in_maps.append(m)

    trace = os.environ.get("KERNEL_TRACE", "") == "1"
    LAST_RESULTS = run_bass_kernel_spmd(nc, in_maps, core_ids=list(range(NCORES)),
                                        trace=trace)
    return np.concatenate([r["out"] for r in LAST_RESULTS.results], axis=0)


# revision 11
# speedup vs baseline: 1.0079x; 1.0079x over previous
"""Trainium2 Bass kernel for nn_Actor (dense MLP trunk + 64 softmax heads).

Data-parallel over 8 NeuronCores: batch 4096 -> 512 rows/core, weights
replicated. Feature-major trunk (activations [features, batch]) so layer
outputs feed the next contraction without transposes; heads run batch-major
so per-head softmax reduces along the free dim.

Precision split: trunk matmuls in bf16 (moving operand streams 2 elem/cycle,
LDWEIGHTS hidden, half the weight DMA), head matmuls in float32r (~1.5e-4)
with the stationary h2 tile reused across 4 column chunks (k-outer loop).

Self-contained: hardcodes shapes; host-side prep packs head weights into one
[1024, 1280] GEMM whose columns are already in the final output order
(per vehicle v: rsu[2v] | rsu[2v+1] | lay[2v] | lay[2v+1]); head bias is
folded in multiplicatively via exp(bias).
"""

import os
import numpy as np

B, IN_DIM, HIDDEN, H2 = 4096, 2048, 2048, 1024
V, R, L = 16, 32, 8
OUTC = V * (2 * R + 2 * L)          # 1280
NCORES = 8
BC = B // NCORES                    # 512 batch rows per core
KT1 = IN_DIM // 128                 # 16 k-tiles, layer 1
MT1 = HIDDEN // 128                 # 16 m-tiles, layer 1
KT2 = HIDDEN // 128                 # 16 k-tiles, layer 2
MT2 = H2 // 128                     # 8 m-tiles, layer 2
KTH = H2 // 128                     # 8 k-tiles, heads
BT = BC // 128                      # 4 batch tiles per core
CW = 320                            # head chunk width = 4 vehicles
NCH = OUTC // CW                    # 4 chunks
VC = CW // (2 * (R + L))            # 4 vehicles per chunk

_CACHE = {}
LAST_RESULTS = None                 # BassKernelResults from the last run


def _build():
    import bass_rust
    import concourse.bacc as bacc
    import concourse.mybir as mybir
    import concourse.tile as tile

    F32, F32R = mybir.dt.float32, mybir.dt.float32r
    BF16 = mybir.dt.bfloat16
    Relu = mybir.ActivationFunctionType.Relu
    Exp = mybir.ActivationFunctionType.Exp
    Copy = mybir.ActivationFunctionType.Copy
    X = mybir.AxisListType.X

    nc = bacc.Bacc("TRN2", target_bir_lowering=False, debug=False,
                   num_devices=NCORES)

    xt = nc.dram_tensor("xt", [128, KT1, BC], BF16, kind="ExternalInput")
    w1t = nc.dram_tensor("w1t", [MT1, 128, KT1, 128], BF16, kind="ExternalInput")
    b1c = nc.dram_tensor("b1c", [128, MT1], F32, kind="ExternalInput")
    w2t = nc.dram_tensor("w2t", [MT2, 128, KT2, 128], BF16, kind="ExternalInput")
    b2c = nc.dram_tensor("b2c", [128, MT2], F32, kind="ExternalInput")
    wht = nc.dram_tensor("wht", [128, KTH, OUTC], F32, kind="ExternalInput")
    ebr = nc.dram_tensor("ebr", [128, OUTC], F32, kind="ExternalInput")
    out = nc.dram_tensor("out", [BC, OUTC], F32, kind="ExternalOutput")

    with tile.TileContext(nc) as tc:
        with (
            tc.tile_pool(name="const", bufs=1) as cp,
            tc.tile_pool(name="wpool", bufs=6) as wp,
            tc.tile_pool(name="sm", bufs=3) as sp,
            tc.tile_pool(name="ps", bufs=4, space="PSUM") as ps,
            tc.tile_pool(name="psh", bufs=2, space="PSUM") as psh,
        ):
            # PE warmup: ~3.5us of dense dummy matmuls so the HAM clock
            # gate opens before the first (DMA-paced, sparse) real matmuls.
            warm = cp.tile([128, 128], BF16, tag="warm")
            nc.gpsimd.memset(warm[:], 0.0)
            wacc = ps.tile([128, 512], F32, tag="acc")
            for i in range(30):
                nc.tensor.matmul(wacc[:, 0:128], warm[:], warm[:],
                                 start=True, stop=True)

            xt_sb = cp.tile([128, KT1, BC], BF16, tag="xt")
            h1_sb = cp.tile([128, MT1, BC], BF16, tag="h1")
            h2_sb = cp.tile([128, MT2, BC], F32R, tag="h2")
            wh_sb = cp.tile([128, KTH, OUTC], F32R, tag="wh")
            eb_sb = cp.tile([128, OUTC], F32, tag="eb")
            b1_sb = cp.tile([128, MT1], F32, tag="b1")
            b2_sb = cp.tile([128, MT2], F32, tag="b2")

            # One HWDGE ring (sync), loads in exact consumption order:
            # biases, then w1[0] quarters interleaved with xt quads so the
            # PE starts after ~300KB and never waits long for either stream.
            nc.sync.dma_start(b1_sb[:], b1c.ap())
            nc.sync.dma_start(b2_sb[:], b2c.ap())
            w1m0 = wp.tile([128, KT1, 128], BF16, tag="wm")
            for q in range(4):
                nc.sync.dma_start(w1m0[:, 4 * q:4 * (q + 1), :],
                                  w1t.ap()[0][:, 4 * q:4 * (q + 1), :])
                for k in range(4 * q, 4 * q + 4):
                    nc.sync.dma_start(xt_sb[:, k, :], xt.ap()[:, k, :])

            # Layer 1: h1[m] = relu(sum_k w1[k,m].T @ xt[k] + b1[m])
            # w1 m-slices stream on the sync ring; first slice split so the
            # PE can start after ~128KB.
            for m in range(MT1):
                if m == 0:
                    w1m = w1m0
                else:
                    w1m = wp.tile([128, KT1, 128], BF16, tag="wm")
                    nc.sync.dma_start(w1m[:], w1t.ap()[m])
                acc = ps.tile([128, 512], F32, tag="acc")
                for k in range(KT1):
                    nc.tensor.matmul(acc[:], w1m[:, k, :], xt_sb[:, k, :],
                                     start=(k == 0), stop=(k == KT1 - 1))
                relu = nc.scalar.activation(h1_sb[:, m, :], acc[:], Relu,
                                            bias=b1_sb[:, m:m + 1], scale=1.0)
                # wh prefetch paced to L1 progress so it can't starve the
                # xt/w1 streams of HBM bandwidth at kernel start
                if m >= 2 and m % 2 == 0:
                    kh = (m - 2) // 2
                    whd = nc.gpsimd.dma_start(wh_sb[:, kh, :],
                                              wht.ap()[:, kh, :].bitcast(F32R))
                    bass_rust.add_dep_helper(whd.ins, relu.ins, sync=True,
                                             reason="pace wh prefetch")
                if m == 15:
                    ebd = nc.gpsimd.dma_start(eb_sb[:], ebr.ap())
                    bass_rust.add_dep_helper(ebd.ins, relu.ins, sync=True,
                                             reason="pace eb prefetch")

            # Layer 2: h2[m] = relu(sum_k w2[k,m].T @ h1[k] + b2[m])
            for m in range(MT2):
                w2m = wp.tile([128, KT2, 128], BF16, tag="wm")
                nc.sync.dma_start(w2m[:], w2t.ap()[m])
                acc = ps.tile([128, 512], F32, tag="acc")
                for k in range(KT2):
                    nc.tensor.matmul(acc[:], w2m[:, k, :], h1_sb[:, k, :],
                                     start=(k == 0), stop=(k == KT2 - 1))
                relu = nc.scalar.activation(h2_sb[:, m, :], acc[:], Relu,
                                            bias=b2_sb[:, m:m + 1], scale=1.0)
                if m == 0:
                    whd = nc.gpsimd.dma_start(wh_sb[:, KTH - 1, :],
                                              wht.ap()[:, KTH - 1, :].bitcast(F32R))
                    bass_rust.add_dep_helper(whd.ins, relu.ins, sync=True,
                                             reason="pace wh prefetch")

            # Heads: logits[b, :] = h2[:, b].T @ wh, f32r, k-outer so each
            # stationary h2 tile is loaded once and reused for all 4 chunks.
            # softmax(l + bias) == exp(l)*exp(bias) / sum(exp(l)*exp(bias))
            for bt in range(BT):
                bsl = slice(bt * 128, (bt + 1) * 128)
                for pr in range(NCH // 2):
                    accs = []
                    for ci in range(2):
                        hacc = psh.tile([128, CW], F32, tag=f"hacc{ci}")
                        accs.append(hacc)
                    for k in range(KTH):
                        for ci in range(2):
                            c = 2 * pr + ci
                            nc.tensor.matmul(accs[ci][:], h2_sb[:, k, bsl],
                                             wh_sb[:, k, c * CW:(c + 1) * CW],
                                             start=(k == 0), stop=(k == KTH - 1))
                    for ci in range(2):
                        c = 2 * pr + ci
                        c0 = c * CW
                        last = (bt == BT - 1)
                        et = sp.tile([128, CW], F32, tag="et")
                        nc.scalar.activation(et[:], accs[ci][:], Exp)
                        num = sp.tile([128, CW], F32, tag="num")
                        mul_eng = nc.vector if last else nc.gpsimd
                        mul_eng.tensor_mul(num[:], et[:], eb_sb[:, c0:c0 + CW])

                        nv = num[:].rearrange("p (v x) -> p v x", v=VC)
                        rsu4 = nv[:, :, 0:2 * R].rearrange("p v (h c) -> p v h c", h=2)
                        lay4 = nv[:, :, 2 * R:].rearrange("p v (h c) -> p v h c", h=2)
                        sums = sp.tile([128, 4 * VC], F32, tag="sums")
                        s_r = sums[:, 0:2 * VC].rearrange("p (v h) -> p v h", h=2)
                        s_l = sums[:, 2 * VC:].rearrange("p (v h) -> p v h", h=2)
                        nc.vector.reduce_sum(out=s_r.unsqueeze(3), in_=rsu4, axis=X)
                        nc.vector.reduce_sum(out=s_l.unsqueeze(3), in_=lay4, axis=X)
                        rec = sp.tile([128, 4 * VC], F32, tag="rec")
                        nc.vector.reciprocal(rec[:], sums[:])

                        o_sb = sp.tile([128, CW], F32, tag="o")
                        ov = o_sb[:].rearrange("p (v x) -> p v x", v=VC)
                        orsu = ov[:, :, 0:2 * R].rearrange("p v (h c) -> p v h c", h=2)
                        olay = ov[:, :, 2 * R:].rearrange("p v (h c) -> p v h c", h=2)
                        r_r = rec[:, 0:2 * VC].rearrange("p (v h) -> p v h", h=2)
                        r_l = rec[:, 2 * VC:].rearrange("p (v h) -> p v h", h=2)
                        nc.vector.tensor_mul(
                            orsu, rsu4, r_r.unsqueeze(3).broadcast_to([128, VC, 2, R]))
                        nc.gpsimd.tensor_mul(
                            olay, lay4, r_l.unsqueeze(3).broadcast_to([128, VC, 2, L]))
                        nc.sync.dma_start(out.ap()[bsl, c0:c0 + CW], o_sb[:])

    nc.compile()
    return nc


def _prep_shared(w1, b1, w2, b2, w_rsu, b_rsu, w_lay, b_lay):
    import ml_dtypes
    f, bf = np.float32, ml_dtypes.bfloat16
    w1t = np.ascontiguousarray(
        w1.astype(bf).reshape(KT1, 128, MT1, 128).transpose(2, 1, 0, 3))
    w2t = np.ascontiguousarray(
        w2.astype(bf).reshape(KT2, 128, MT2, 128).transpose(2, 1, 0, 3))
    b1c = np.ascontiguousarray(b1.reshape(MT1, 128).T, dtype=f)
    b2c = np.ascontiguousarray(b2.reshape(MT2, 128).T, dtype=f)

    wh = np.empty((H2, OUTC), dtype=f)
    bh = np.empty((OUTC,), dtype=f)
    for v in range(V):
        c = 2 * (R + L) * v
        wh[:, c:c + R] = w_rsu[2 * v]
        wh[:, c + R:c + 2 * R] = w_rsu[2 * v + 1]
        wh[:, c + 2 * R:c + 2 * R + L] = w_lay[2 * v]
        wh[:, c + 2 * R + L:c + 2 * (R + L)] = w_lay[2 * v + 1]
        bh[c:c + R] = b_rsu[2 * v]
        bh[c + R:c + 2 * R] = b_rsu[2 * v + 1]
        bh[c + 2 * R:c + 2 * R + L] = b_lay[2 * v]
        bh[c + 2 * R + L:c + 2 * (R + L)] = b_lay[2 * v + 1]
    wht = np.ascontiguousarray(
        wh.reshape(KTH, 128, OUTC).transpose(1, 0, 2), dtype=f)
    ebr = np.ascontiguousarray(
        np.broadcast_to(np.exp(bh)[None, :], (128, OUTC)), dtype=f)
    return {"w1t": w1t, "b1c": b1c, "w2t": w2t, "b2c": b2c,
            "wht": wht, "ebr": ebr}


def kernel(x, w1, b1, w2, b2, w_rsu, b_rsu, w_lay, b_lay):
    global LAST_RESULTS
    import ml_dtypes
    from concourse.bass_utils import run_bass_kernel_spmd

    if "nc" not in _CACHE:
        _CACHE["nc"] = _build()
    nc = _CACHE["nc"]

    shared = _prep_shared(np.asarray(w1, np.float32), np.asarray(b1, np.float32),
                          np.asarray(w2, np.float32), np.asarray(b2, np.float32),
                          np.asarray(w_rsu, np.float32), np.asarray(b_rsu, np.float32),
                          np.asarray(w_lay, np.float32), np.asarray(b_lay, np.float32))

    # x [B, IN] -> per-core xt [128, KT1, BC] with [p, k, n] = x[core*BC+n, k*128+p]
    xt_full = np.ascontiguousarray(np.asarray(x, np.float32).T) \
        .astype(ml_dtypes.bfloat16).reshape(KT1, 128, B).transpose(1, 0, 2)
    in_maps = []
    for c in range(NCORES):
        m = dict(shared)
        m["xt"] = np.ascontiguousarray(xt_full[:, :, c * BC:(c + 1) * BC])
        in_maps.append(m)

    trace = os.environ.get("KERNEL_TRACE", "") == "1"
    LAST_RESULTS = run_bass_kernel_spmd(nc, in_maps, core_ids=list(range(NCORES)),
                                        trace=trace)
    return np.concatenate([r["out"] for r in LAST_RESULTS.results], axis=0)


# revision 12
# speedup vs baseline: 1.0148x; 1.0068x over previous
"""Trainium2 Bass kernel for nn_Actor (dense MLP trunk + 64 softmax heads).

Data-parallel over 8 NeuronCores: batch 4096 -> 512 rows/core, weights
replicated. Feature-major trunk (activations [features, batch]) so layer
outputs feed the next contraction without transposes; heads run batch-major
so per-head softmax reduces along the free dim.

Precision split: trunk matmuls in bf16 (moving operand streams 2 elem/cycle,
LDWEIGHTS hidden, half the weight DMA), head matmuls in float32r (~1.5e-4)
with the stationary h2 tile reused across 4 column chunks (k-outer loop).

Self-contained: hardcodes shapes; host-side prep packs head weights into one
[1024, 1280] GEMM whose columns are already in the final output order
(per vehicle v: rsu[2v] | rsu[2v+1] | lay[2v] | lay[2v+1]); head bias is
folded in multiplicatively via exp(bias).
"""

import os
import numpy as np

B, IN_DIM, HIDDEN, H2 = 4096, 2048, 2048, 1024
V, R, L = 16, 32, 8
OUTC = V * (2 * R + 2 * L)          # 1280
NCORES = 8
BC = B // NCORES                    # 512 batch rows per core
KT1 = IN_DIM // 128                 # 16 k-tiles, layer 1
MT1 = HIDDEN // 128                 # 16 m-tiles, layer 1
KT2 = HIDDEN // 128                 # 16 k-tiles, layer 2
MT2 = H2 // 128                     # 8 m-tiles, layer 2
KTH = H2 // 128                     # 8 k-tiles, heads
BT = BC // 128                      # 4 batch tiles per core
CW = 320                            # head chunk width = 4 vehicles
NCH = OUTC // CW                    # 4 chunks
VC = CW // (2 * (R + L))            # 4 vehicles per chunk

_CACHE = {}
LAST_RESULTS = None                 # BassKernelResults from the last run


def _build():
    import bass_rust
    import concourse.bacc as bacc
    import concourse.mybir as mybir
    import concourse.tile as tile

    F32, F32R = mybir.dt.float32, mybir.dt.float32r
    BF16 = mybir.dt.bfloat16
    Relu = mybir.ActivationFunctionType.Relu
    Exp = mybir.ActivationFunctionType.Exp
    Copy = mybir.ActivationFunctionType.Copy
    X = mybir.AxisListType.X

    nc = bacc.Bacc("TRN2", target_bir_lowering=False, debug=False,
                   num_devices=NCORES)

    xt = nc.dram_tensor("xt", [128, KT1, BC], BF16, kind="ExternalInput")
    w1t = nc.dram_tensor("w1t", [MT1, 128, KT1, 128], BF16, kind="ExternalInput")
    b1c = nc.dram_tensor("b1c", [128, MT1], F32, kind="ExternalInput")
    w2t = nc.dram_tensor("w2t", [MT2, 128, KT2, 128], BF16, kind="ExternalInput")
    b2c = nc.dram_tensor("b2c", [128, MT2], F32, kind="ExternalInput")
    wht = nc.dram_tensor("wht", [128, KTH, OUTC], F32, kind="ExternalInput")
    ebr = nc.dram_tensor("ebr", [128, OUTC], F32, kind="ExternalInput")
    out = nc.dram_tensor("out", [BC, OUTC], F32, kind="ExternalOutput")

    with tile.TileContext(nc) as tc:
        with (
            tc.tile_pool(name="const", bufs=1) as cp,
            tc.tile_pool(name="wpool", bufs=6) as wp,
            tc.tile_pool(name="sm", bufs=3) as sp,
            tc.tile_pool(name="ps", bufs=4, space="PSUM") as ps,
            tc.tile_pool(name="psh", bufs=2, space="PSUM") as psh,
        ):
            # PE warmup: ~3.5us of dense dummy matmuls so the HAM clock
            # gate opens before the first (DMA-paced, sparse) real matmuls.
            warm = cp.tile([128, 128], BF16, tag="warm")
            nc.gpsimd.memset(warm[:], 0.0)
            wacc = ps.tile([128, 512], F32, tag="acc")
            for i in range(30):
                nc.tensor.matmul(wacc[:, 0:128], warm[:], warm[:],
                                 start=True, stop=True)

            xt_sb = cp.tile([128, KT1, BC], BF16, tag="xt")
            h1_sb = cp.tile([128, MT1, BC], BF16, tag="h1")
            h2_sb = cp.tile([128, MT2, BC], F32R, tag="h2")
            wh_sb = cp.tile([128, KTH, OUTC], F32R, tag="wh")
            eb_sb = cp.tile([128, OUTC], F32, tag="eb")
            b1_sb = cp.tile([128, MT1], F32, tag="b1")
            b2_sb = cp.tile([128, MT2], F32, tag="b2")

            # One HWDGE ring (sync), loads in exact consumption order:
            # biases, then w1[0] quarters interleaved with xt quads so the
            # PE starts after ~300KB and never waits long for either stream.
            nc.sync.dma_start(b1_sb[:], b1c.ap())
            nc.sync.dma_start(b2_sb[:], b2c.ap())
            w1m0 = wp.tile([128, KT1, 128], BF16, tag="wm")
            for q in range(4):
                nc.sync.dma_start(w1m0[:, 4 * q:4 * (q + 1), :],
                                  w1t.ap()[0][:, 4 * q:4 * (q + 1), :])
                for k in range(4 * q, 4 * q + 4):
                    nc.sync.dma_start(xt_sb[:, k, :], xt.ap()[:, k, :])

            # Layer 1: h1[m] = relu(sum_k w1[k,m].T @ xt[k] + b1[m])
            # w1 m-slices stream on the sync ring; first slice split so the
            # PE can start after ~128KB.
            for m in range(MT1):
                if m == 0:
                    w1m = w1m0
                else:
                    w1m = wp.tile([128, KT1, 128], BF16, tag="wm")
                    nc.sync.dma_start(w1m[:], w1t.ap()[m])
                acc = ps.tile([128, 512], F32, tag="acc")
                for k in range(KT1):
                    nc.tensor.matmul(acc[:], w1m[:, k, :], xt_sb[:, k, :],
                                     start=(k == 0), stop=(k == KT1 - 1))
                relu = nc.scalar.activation(h1_sb[:, m, :], acc[:], Relu,
                                            bias=b1_sb[:, m:m + 1], scale=1.0)
                # wh prefetch paced to L1 progress so it can't starve the
                # xt/w1 streams of HBM bandwidth at kernel start
                if m >= 2 and m % 2 == 0:
                    kh = (m - 2) // 2
                    whd = nc.gpsimd.dma_start(wh_sb[:, kh, :],
                                              wht.ap()[:, kh, :].bitcast(F32R))
                    bass_rust.add_dep_helper(whd.ins, relu.ins, sync=True,
                                             reason="pace wh prefetch")
                if m == 15:
                    ebd = nc.gpsimd.dma_start(eb_sb[:], ebr.ap())
                    bass_rust.add_dep_helper(ebd.ins, relu.ins, sync=True,
                                             reason="pace eb prefetch")

            # Layer 2: h2[m] = relu(sum_k w2[k,m].T @ h1[k] + b2[m])
            for m in range(MT2):
                w2m = wp.tile([128, KT2, 128], BF16, tag="wm")
                nc.sync.dma_start(w2m[:], w2t.ap()[m])
                acc = ps.tile([128, 512], F32, tag="acc")
                for k in range(KT2):
                    nc.tensor.matmul(acc[:], w2m[:, k, :], h1_sb[:, k, :],
                                     start=(k == 0), stop=(k == KT2 - 1))
                relu = nc.scalar.activation(h2_sb[:, m, :], acc[:], Relu,
                                            bias=b2_sb[:, m:m + 1], scale=1.0)
                if m == 0:
                    whd = nc.gpsimd.dma_start(wh_sb[:, KTH - 1, :],
                                              wht.ap()[:, KTH - 1, :].bitcast(F32R))
                    bass_rust.add_dep_helper(whd.ins, relu.ins, sync=True,
                                             reason="pace wh prefetch")

            # Heads: logits[b, :] = h2[:, b].T @ wh, f32r, k-outer so each
            # stationary h2 tile is loaded once and reused for all 4 chunks.
            # softmax(l + bias) == exp(l)*exp(bias) / sum(exp(l)*exp(bias))
            for bt in range(BT):
                bsl = slice(bt * 128, (bt + 1) * 128)
                for pr in range(NCH // 2):
                    accs = []
                    for ci in range(2):
                        hacc = psh.tile([128, CW], F32, tag=f"hacc{ci}")
                        accs.append(hacc)
                    for k in range(KTH):
                        for ci in range(2):
                            c = 2 * pr + ci
                            nc.tensor.matmul(accs[ci][:], h2_sb[:, k, bsl],
                                             wh_sb[:, k, c * CW:(c + 1) * CW],
                                             start=(k == 0), stop=(k == KTH - 1))
                    # pair-level softmax on [128, 640] so per-op overhead
                    # is paid once per pair instead of once per chunk
                    c0 = 2 * pr * CW
                    PW = 2 * CW
                    VP = 2 * VC
                    et = sp.tile([128, PW], F32, tag="et")
                    for ci in range(2):
                        nc.scalar.activation(et[:, ci * CW:(ci + 1) * CW],
                                             accs[ci][:], Exp)
                    num = sp.tile([128, PW], F32, tag="num")
                    nc.vector.tensor_mul(num[:], et[:], eb_sb[:, c0:c0 + PW])

                    nv = num[:].rearrange("p (v x) -> p v x", v=VP)
                    rsu4 = nv[:, :, 0:2 * R].rearrange("p v (h c) -> p v h c", h=2)
                    lay4 = nv[:, :, 2 * R:].rearrange("p v (h c) -> p v h c", h=2)
                    sums = sp.tile([128, 4 * VP], F32, tag="sums")
                    s_r = sums[:, 0:2 * VP].rearrange("p (v h) -> p v h", h=2)
                    s_l = sums[:, 2 * VP:].rearrange("p (v h) -> p v h", h=2)
                    nc.vector.reduce_sum(out=s_r.unsqueeze(3), in_=rsu4, axis=X)
                    nc.vector.reduce_sum(out=s_l.unsqueeze(3), in_=lay4, axis=X)
                    rec = sp.tile([128, 4 * VP], F32, tag="rec")
                    nc.vector.reciprocal(rec[:], sums[:])

                    o_sb = sp.tile([128, PW], F32, tag="o")
                    ov = o_sb[:].rearrange("p (v x) -> p v x", v=VP)
                    orsu = ov[:, :, 0:2 * R].rearrange("p v (h c) -> p v h c", h=2)
                    olay = ov[:, :, 2 * R:].rearrange("p v (h c) -> p v h c", h=2)
                    r_r = rec[:, 0:2 * VP].rearrange("p (v h) -> p v h", h=2)
                    r_l = rec[:, 2 * VP:].rearrange("p (v h) -> p v h", h=2)
                    nc.gpsimd.tensor_mul(
                        orsu, rsu4, r_r.unsqueeze(3).broadcast_to([128, VP, 2, R]))
                    nc.vector.tensor_mul(
                        olay, lay4, r_l.unsqueeze(3).broadcast_to([128, VP, 2, L]))
                    nc.sync.dma_start(out.ap()[bsl, c0:c0 + PW], o_sb[:])

    nc.compile()
    return nc


def _prep_shared(w1, b1, w2, b2, w_rsu, b_rsu, w_lay, b_lay):
    import ml_dtypes
    f, bf = np.float32, ml_dtypes.bfloat16
    w1t = np.ascontiguousarray(
        w1.astype(bf).reshape(KT1, 128, MT1, 128).transpose(2, 1, 0, 3))
    w2t = np.ascontiguousarray(
        w2.astype(bf).reshape(KT2, 128, MT2, 128).transpose(2, 1, 0, 3))
    b1c = np.ascontiguousarray(b1.reshape(MT1, 128).T, dtype=f)
    b2c = np.ascontiguousarray(b2.reshape(MT2, 128).T, dtype=f)

    wh = np.empty((H2, OUTC), dtype=f)
    bh = np.empty((OUTC,), dtype=f)
    for v in range(V):
        c = 2 * (R + L) * v
        wh[:, c:c + R] = w_rsu[2 * v]
        wh[:, c + R:c + 2 * R] = w_rsu[2 * v + 1]
        wh[:, c + 2 * R:c + 2 * R + L] = w_lay[2 * v]
        wh[:, c + 2 * R + L:c + 2 * (R + L)] = w_lay[2 * v + 1]
        bh[c:c + R] = b_rsu[2 * v]
        bh[c + R:c + 2 * R] = b_rsu[2 * v + 1]
        bh[c + 2 * R:c + 2 * R + L] = b_lay[2 * v]
        bh[c + 2 * R + L:c + 2 * (R + L)] = b_lay[2 * v + 1]
    wht = np.ascontiguousarray(
        wh.reshape(KTH, 128, OUTC).transpose(1, 0, 2), dtype=f)
    ebr = np.ascontiguousarray(
        np.broadcast_to(np.exp(bh)[None, :], (128, OUTC)), dtype=f)
    return {"w1t": w1t, "b1c": b1c, "w2t": w2t, "b2c": b2c,
            "wht": wht, "ebr": ebr}


def kernel(x, w1, b1, w2, b2, w_rsu, b_rsu, w_lay, b_lay):
    global LAST_RESULTS
    import ml_dtypes
    from concourse.bass_utils import run_bass_kernel_spmd

    if "nc" not in _CACHE:
        _CACHE["nc"] = _build()
    nc = _CACHE["nc"]

    shared = _prep_shared(np.asarray(w1, np.float32), np.asarray(b1, np.float32),
                          np.asarray(w2, np.float32), np.asarray(b2, np.float32),
                          np.asarray(w_rsu, np.float32), np.asarray(b_rsu, np.float32),
                          np.asarray(w_lay, np.float32), np.asarray(b_lay, np.float32))

    # x [B, IN] -> per-core xt [128, KT1, BC] with [p, k, n] = x[core*BC+n, k*128+p]
    xt_full = np.ascontiguousarray(np.asarray(x, np.float32).T) \
        .astype(ml_dtypes.bfloat16).reshape(KT1, 128, B).transpose(1, 0, 2)
    in_maps = []
    for c in range(NCORES):
        m = dict(shared)
        m["xt"] = np.ascontiguousarray(xt_full[:, :, c * BC:(c + 1) * BC])
        in_maps.append(m)

    trace = os.environ.get("KERNEL_TRACE", "") == "1"
    LAST_RESULTS = run_bass_kernel_spmd(nc, in_maps, core_ids=list(range(NCORES)),
                                        trace=trace)
    return np.concatenate([r["out"] for r in LAST_RESULTS.results], axis=0)
